# revision 7
# baseline (speedup 1.0000x reference)
"""Trainium2 Bass kernel (raw bass, hand-pipelined) for additive attention.

Math per batch b (1024 batches, 128 per core, pure data parallel, 8 cores):
    f   = features[b] @ W1_w.T               # [seq=360, units=512]
    r   = relu(hidden[b] @ W2_w.T + W2_b)    # [units]
    a   = tanh(relu(f + W1_b) + r) = max(tanh(f + W1_b + r), tanh(r))
    s   = a @ V_w.T        (V_b dropped: softmax is shift-invariant)
    w   = softmax(s); ctx = w @ features[b]
returns (ctx [1024, 512] f32, w [1024, 360, 1] f32)

Raw-bass engine streams with manual semaphores (the bundled Tile scheduler
emits multi-wait instructions this walrus rejects). Steady-state schedule,
one "iteration" i per batch (groups g of 4 batches; q = g%2 slot):

  SP  iter i: outputs(g=(i-12)/4) | loads(i+2) | xbar transposes(i)
  DVE iter i: cast(i) | ps-memset((i-1)/4) | A'(i-2) | preadd(i-2)
              | softmaxA((i-6)/4) | softmaxB+wt-copies((i-8)/4)
              | cs-copy((i-10)/4)
  ACT iter i: exp((i-7)/4) | tanh(i) x4
  PE  iter i: main-mm(i) | score(i-2) | wb-transposes((i-8)/4) | ctx((i-9)/4)

Main matmul: out[u,s] = W1T.T @ FT in bf16, N=360 (FT from DMA-xbar
transpose of the DVE-cast bf16 features). Score reduce via ones-vector
matmul into per-batch PSUM partitions 32j (col-group packed, 4 batches /
bank). Context matmuls 4-way col-packed against the bf16 natural features.
"""

import sys
import numpy as np

sys.path.insert(0, "/opt/trn_rl_repo")

import concourse.bass as bass  # noqa: E402
from concourse import mybir  # noqa: E402
from concourse.bass_utils import run_bass_kernel_spmd  # noqa: E402

F32 = mybir.dt.float32
BF16 = mybir.dt.bfloat16
ALU = mybir.AluOpType
ACTF = mybir.ActivationFunctionType
AX = mybir.AxisListType

N_CORES = 8
BS = 1024
B = BS // N_CORES  # 128
SEQ = 360
EMB = 512
HID = 512
UNITS = 512
G = B // 4  # 32 groups

S_CHUNKS = [(0, 128, 128), (128, 128, 128), (256, 104, 112)]  # (s0, rows, xbar_rows)
KC = 4
MC = 4

NF = 3    # f32 feature slots
NB = 12   # bf16 feature slots
NT = 3    # transposed feature slots
NE = 4    # y/x/xs slots

OUT_W = EMB + SEQ
FTW = 368
VIT = 140  # virtual iterations (B + drain tail)


def build_nc():
    nc = bass.Bass(trn_type="TRN2", target_bir_lowering=False, debug=False)

    feat = nc.declare_dram_parameter("features", [B, SEQ, EMB], F32, isOutput=False).ap()
    hid = nc.declare_dram_parameter("hidden", [B, HID], F32, isOutput=False).ap()
    w1t = nc.declare_dram_parameter("w1t", [EMB, UNITS], F32, isOutput=False).ap()
    w2t = nc.declare_dram_parameter("w2t", [HID, UNITS], F32, isOutput=False).ap()
    vby = nc.declare_dram_parameter("vby", [128, 12], F32, isOutput=False).ap()
    ident = nc.declare_dram_parameter("ident", [128, 128], F32, isOutput=False).ap()
    out = nc.declare_dram_parameter("out", [B, OUT_W], F32, isOutput=True).ap()
    # vby cols: 0-3 V chunks, 4-7 W1_b chunks, 8-11 W2_b chunks

    A = lambda name, shape, dt=F32: nc.alloc_sbuf_tensor(name, shape, dt).ap()

    id_sb = A("id_sb", [128, 128])
    vby_sb = A("vby_sb", [128, 12])
    w1t_f = [A(f"w1tf{k}", [128, UNITS]) for k in range(KC)]
    w1t_b = [A(f"w1tb{k}", [128, UNITS], BF16) for k in range(KC)]
    w2t_f = [A(f"w2tf{k}", [128, UNITS]) for k in range(KC)]
    hid_sb = A("hid_sb", [128, HID])
    ht = [A(f"ht{k}", [128, 128]) for k in range(KC)]
    r_sb = [A(f"r{m}", [128, B]) for m in range(MC)]
    rb1 = [A(f"rb1{m}", [128, B]) for m in range(MC)]
    thr = [A(f"thr{m}", [128, B]) for m in range(MC)]
    ones_bf = A("ones_bf", [128, 1], BF16)

    fn = [[A(f"fn{c}_{a}", [128, EMB]) for a in range(NF)] for c in range(3)]
    fb = [[A(f"fb{c}_{a}", [128, EMB], BF16) for a in range(NB)] for c in range(3)]
    ft = [A(f"ft{d}", [128, KC * FTW], BF16) for d in range(NT)]
    y = [[A(f"y{m}_{e}", [128, SEQ], BF16) for e in range(NE)] for m in range(MC)]
    x = [[A(f"x{m}_{e}", [128, SEQ], BF16) for e in range(NE)] for m in range(MC)]
    xs = [A(f"xs{e}", [128, SEQ], BF16) for e in range(NE)]
    mx = [A(f"mx{q}", [97, 1]) for q in range(2)]
    nmx = [A(f"nmx{q}", [97, 1]) for q in range(2)]
    se = [A(f"se{q}", [97, 1]) for q in range(2)]
    rs = [A(f"rs{q}", [97, 1]) for q in range(2)]
    eb = [A(f"eb{q}", [97, SEQ]) for q in range(2)]
    wb = [A(f"wb{q}", [97, SEQ]) for q in range(2)]
    wt = [[A(f"wt{c}_{q}", [128, 97], BF16) for q in range(2)] for c in range(3)]
    cs = [A(f"cs{q}", [97, EMB]) for q in range(2)]

    P = lambda name, shape: nc.alloc_psum_tensor(name, shape, F32).ap()
    pf = [P(f"pf{m}", [128, SEQ]) for m in range(MC)]
    ps = [P(f"ps{q}", [97, SEQ]) for q in range(2)]
    pc = P("pc", [97, EMB])
    pt = P("pt", [128, 128])

    S = lambda name: nc.alloc_semaphore(name)
    s_prep = S("s_prep")
    s_ppe = S("s_ppe")
    s_ppv = S("s_ppv")
    s_phm = S("s_phm")
    s_pact = S("s_pact")
    s_prb = S("s_prb")
    s_load = S("s_load")
    s_cast = S("s_cast")
    s_tp = S("s_tp")
    s_mm = S("s_mm")
    s_tanh = S("s_tanh")
    s_sum = S("s_sum")
    s_pmz = S("s_pmz")
    s_sc = S("s_sc")
    s_max = S("s_max")
    s_exp = S("s_exp")
    s_soft = S("s_soft")
    s_wtp = S("s_wtp")
    s_wtc = S("s_wtc")
    s_ctx = S("s_ctx")
    s_csc = S("s_csc")
    s_wout = S("s_wout")

    N_PREP_DMAS = 11

    with nc.Block() as block:

        @block.sync
        def _(sync):
            sync.dma_start(out=id_sb, in_=ident).then_inc(s_prep, 16)
            sync.dma_start(out=vby_sb, in_=vby).then_inc(s_prep, 16)
            for k in range(KC):
                sync.dma_start(out=w1t_f[k], in_=w1t[k * 128:(k + 1) * 128, :]).then_inc(s_prep, 16)
                sync.dma_start(out=w2t_f[k], in_=w2t[k * 128:(k + 1) * 128, :]).then_inc(s_prep, 16)
            sync.dma_start(out=hid_sb, in_=hid).then_inc(s_prep, 16)

            # prologue loads for batches 0, 1
            for b0 in (0, 1):
                for c, (s0, rows, _) in enumerate(S_CHUNKS):
                    sync.dma_start(
                        out=fn[c][b0 % NF][:rows, :], in_=feat[b0, s0:s0 + rows, :]
                    ).then_inc(s_load, 16)

            for i in range(VIT):
                # outputs for group (i-12)//4
                if i % 4 == 0 and i >= 12 and (i - 12) // 4 < G:
                    go = (i - 12) // 4
                    sync.wait_ge(s_soft, go + 1)
                    for j in range(4):
                        bg = 4 * go + j
                        sync.dma_start(
                            out=out[bg:bg + 1, EMB:EMB + SEQ],
                            in_=wb[go % 2][32 * j:32 * j + 1, :],
                        ).then_inc(s_wout, 16)
                    sync.wait_ge(s_csc, go + 1)
                    for j in range(4):
                        bg = 4 * go + j
                        sync.dma_start(
                            out=out[bg:bg + 1, 0:EMB],
                            in_=cs[go % 2][32 * j:32 * j + 1, :],
                        ).then_inc(s_wout, 16)
                # loads for batch i+2
                if i + 2 < B:
                    if i >= 1:
                        sync.wait_ge(s_cast, i)  # fn slot WAR vs cast(i-1)
                    for c, (s0, rows, _) in enumerate(S_CHUNKS):
                        sync.dma_start(
                            out=fn[c][(i + 2) % NF][:rows, :],
                            in_=feat[i + 2, s0:s0 + rows, :],
                        ).then_inc(s_load, 16)
                # xbar transposes for batch i
                if i < B:
                    sync.wait_ge(s_cast, i + 1)
                    if i >= NT:
                        sync.wait_ge(s_mm, 4 * (i - NT) + 4)
                    for c, (s0, rows, prows) in enumerate(S_CHUNKS):
                        for k in range(KC):
                            sync.dma_start(
                                out=ft[i % NT][:, k * FTW + s0:k * FTW + s0 + prows],
                                in_=fb[c][i % NB][:prows, k * 128:(k + 1) * 128],
                                transpose=True,
                            ).then_inc(s_tp, 16)
            sync.wait_ge(s_wout, 128 * G)

        @block.vector
        def _(vector):
            # prep: hT copies (ping-pong over pt with PE)
            for k in range(KC):
                vector.wait_ge(s_ppe, k + 1)
                vector.tensor_copy(ht[k], pt[:, :128]).then_inc(s_ppv, 1)
            for k in range(KC):
                vector.tensor_copy(w1t_b[k], w1t_f[k])
            vector.memset(ones_bf, 1.0)
            for m in range(MC):
                vector.wait_ge(s_pact, m + 1)
                vector.tensor_scalar_add(
                    rb1[m], r_sb[m], vby_sb[:, 4 + m:5 + m]
                ).then_inc(s_prb, 1)

            for i in range(VIT):
                # cast(i)
                if i < B:
                    vector.wait_ge(s_load, 48 * (i + 1))
                    if i >= NB:
                        vector.wait_ge(s_tp, 192 * (i - NB + 1))
                        vector.wait_ge(s_ctx, (i - NB) // 4 + 1)
                    for c, (s0, rows, _) in enumerate(S_CHUNKS):
                        ins = vector.tensor_copy(
                            fb[c][i % NB][:rows, :], fn[c][i % NF][:rows, :]
                        )
                        if c == 2:
                            ins.then_inc(s_cast, 1)
                # ps memset for group (i-1)//4
                if i % 4 == 1 and (i - 1) // 4 < G:
                    gz = (i - 1) // 4
                    if gz >= 2:
                        vector.wait_ge(s_exp, gz - 1)
                    vector.memset(ps[gz % 2], 0.0).then_inc(s_pmz, 1)
                # A'(i-2) + preadd(i-2)
                if 2 <= i < B + 2:
                    b = i - 2
                    for m in range(MC):
                        vector.wait_ge(s_tanh, 4 * b + m + 1)
                        vector.tensor_scalar(
                            out=x[m][b % NE],
                            in0=y[m][b % NE],
                            scalar1=thr[m][:, b:b + 1],
                            scalar2=vby_sb[:, m:m + 1],
                            op0=ALU.max,
                            op1=ALU.mult,
                        )
                    if b >= NE:
                        vector.wait_ge(s_sc, b - NE + 1)  # xs slot WAR
                    vector.tensor_add(xs[b % NE], x[0][b % NE], x[1][b % NE])
                    vector.tensor_add(xs[b % NE], xs[b % NE], x[2][b % NE])
                    vector.tensor_add(
                        xs[b % NE], xs[b % NE], x[3][b % NE]
                    ).then_inc(s_sum, 1)
                # softmax A for group (i-6)//4
                if i % 4 == 2 and i >= 6 and (i - 6) // 4 < G:
                    ga = (i - 6) // 4
                    vector.wait_ge(s_sc, 4 * ga + 4)
                    if ga >= 2:
                        vector.wait_ge(s_exp, ga - 1)  # mx/nmx slot WAR
                    vector.tensor_reduce(mx[ga % 2], ps[ga % 2], AX.X, ALU.max)
                    vector.drain()
                    vector.tensor_scalar_mul(nmx[ga % 2], mx[ga % 2], -1.0).then_inc(
                        s_max, 1
                    )
                # softmax B + wt copies for group (i-8)//4
                if i % 4 == 0 and i >= 8 and (i - 8) // 4 < G:
                    gb = (i - 8) // 4
                    vector.wait_ge(s_exp, gb + 1)
                    if gb >= 2:
                        vector.wait_ge(s_wout, 128 * (gb - 1))  # wb slot WAR
                    vector.reciprocal(rs[gb % 2], se[gb % 2])
                    vector.drain()
                    vector.tensor_scalar_mul(
                        wb[gb % 2], eb[gb % 2], rs[gb % 2]
                    ).then_inc(s_soft, 1)
                    if gb >= 2:
                        vector.wait_ge(s_ctx, gb - 1)  # wt slot WAR
                    for c, (s0, rows, _) in enumerate(S_CHUNKS):
                        vector.wait_ge(s_wtp, 3 * gb + c + 1)
                        vector.tensor_copy(
                            wt[c][gb % 2][:rows, :], pt[:rows, :97]
                        ).then_inc(s_wtc, 1)
                # cs copy for group (i-10)//4
                if i % 4 == 2 and i >= 10 and (i - 10) // 4 < G:
                    gcs = (i - 10) // 4
                    vector.wait_ge(s_ctx, gcs + 1)
                    if gcs >= 2:
                        vector.wait_ge(s_wout, 128 * (gcs - 1))  # cs slot WAR
                    vector.tensor_copy(cs[gcs % 2], pc).then_inc(s_csc, 1)

        @block.scalar
        def _(scalar):
            for m in range(MC):
                scalar.wait_ge(s_phm, m + 1)
                scalar.activation(
                    r_sb[m], pf[m][:, :B], ACTF.Relu,
                    bias=vby_sb[:, 8 + m:9 + m], scale=1.0,
                )
                scalar.activation(thr[m], r_sb[m], ACTF.Tanh).then_inc(s_pact, 1)

            for i in range(VIT):
                # exp for group (i-7)//4
                if i % 4 == 3 and i >= 7 and (i - 7) // 4 < G:
                    ge = (i - 7) // 4
                    scalar.wait_ge(s_max, ge + 1)
                    if ge >= 2:
                        scalar.wait_ge(s_soft, ge - 1)  # eb/se slot WAR
                    scalar.activation(
                        eb[ge % 2], ps[ge % 2], ACTF.Exp,
                        bias=nmx[ge % 2], scale=1.0, accum_out=se[ge % 2],
                    ).then_inc(s_exp, 1)
                # tanh(i)
                if i < B:
                    b = i
                    if b == 0:
                        scalar.wait_ge(s_prb, 4)
                    if b >= NE:
                        scalar.wait_ge(s_sum, b - NE + 1)  # y slot WAR
                    for m in range(MC):
                        scalar.wait_ge(s_mm, 4 * b + m + 1)
                        scalar.activation(
                            y[m][b % NE], pf[m], ACTF.Tanh,
                            bias=rb1[m][:, b:b + 1], scale=1.0,
                        ).then_inc(s_tanh, 1)

        @block.tensor
        def _(tensor):
            tensor.wait_ge(s_prep, 16 * N_PREP_DMAS)
            for k in range(KC):
                if k > 0:
                    tensor.wait_ge(s_ppv, k)
                tensor.transpose(
                    pt[:, :128], hid_sb[:, k * 128:(k + 1) * 128], id_sb
                ).then_inc(s_ppe, 1)
            tensor.wait_ge(s_ppv, 4)
            for m in range(MC):
                for k in range(KC):
                    ins = tensor.matmul(
                        pf[m][:, :B],
                        lhsT=w2t_f[k][:, m * 128:(m + 1) * 128],
                        rhs=ht[k],
                        start=(k == 0),
                        stop=(k == KC - 1),
                    )
                    if k == KC - 1:
                        ins.then_inc(s_phm, 1)

            for i in range(VIT):
                # main matmul(i)
                if i < B:
                    b = i
                    tensor.wait_ge(s_tp, 192 * (b + 1))
                    if b == 0:
                        tensor.wait_ge(s_prb, 1)  # w1t_b + ones done (DVE order)
                    for m in range(MC):
                        if b >= 1:
                            tensor.wait_ge(s_tanh, 4 * (b - 1) + m + 1)
                        else:
                            tensor.wait_ge(s_pact, m + 1)
                        for k in range(KC):
                            ins = tensor.matmul(
                                pf[m],
                                lhsT=w1t_b[k][:, m * 128:(m + 1) * 128],
                                rhs=ft[b % NT][:, k * FTW:k * FTW + SEQ],
                                start=(k == 0),
                                stop=(k == KC - 1),
                            )
                            if k == KC - 1:
                                ins.then_inc(s_mm, 1)
                # score(i-2)
                if 2 <= i < B + 2:
                    b = i - 2
                    g, j = b // 4, b % 4
                    tensor.wait_ge(s_sum, b + 1)
                    tensor.wait_ge(s_pmz, g + 1)
                    tensor.matmul(
                        ps[g % 2][32 * j:32 * j + 1, :],
                        lhsT=ones_bf,
                        rhs=xs[b % NE],
                        start=True,
                        stop=True,
                        tile_position=(0, 32 * j),
                        skip_group_check=True,
                    ).then_inc(s_sc, 1)
                # wb transposes for group (i-8)//4
                if i % 4 == 0 and i >= 8 and (i - 8) // 4 < G:
                    gw = (i - 8) // 4
                    tensor.wait_ge(s_soft, gw + 1)
                    for c, (s0, rows, _) in enumerate(S_CHUNKS):
                        tensor.wait_ge(s_wtc, 3 * gw + c)  # pt WAR
                        tensor.transpose(
                            pt[:rows, :97],
                            wb[gw % 2][:, s0:s0 + rows],
                            id_sb[:97, :97],
                        ).then_inc(s_wtp, 1)
                # context matmuls for group (i-9)//4
                if i % 4 == 1 and i >= 9 and (i - 9) // 4 < G:
                    gc = (i - 9) // 4
                    tensor.wait_ge(s_wtc, 3 * gc + 3)
                    if gc >= 1:
                        tensor.wait_ge(s_csc, gc)  # pc WAR
                    for jj in range(4):
                        bg = 4 * gc + jj
                        for c, (s0, rows, _) in enumerate(S_CHUNKS):
                            ins = tensor.matmul(
                                pc[32 * jj:32 * jj + 1, :],
                                lhsT=wt[c][gc % 2][:rows, 32 * jj:32 * jj + 1],
                                rhs=fb[c][bg % NB][:rows, :],
                                start=(c == 0),
                                stop=(c == 2),
                                tile_position=(0, 32 * jj),
                                skip_group_check=True,
                            )
                            if jj == 3 and c == 2:
                                ins.then_inc(s_ctx, 1)

    return nc


_NC_CACHE = None


def _get_nc():
    global _NC_CACHE
    if _NC_CACHE is None:
        _NC_CACHE = build_nc()
    return _NC_CACHE


def _make_in_maps(inputs):
    hidden = np.asarray(inputs["hidden"], np.float32)
    features = np.asarray(inputs["features"], np.float32)
    w1w = np.asarray(inputs["W1_w"], np.float32)
    w1b = np.asarray(inputs["W1_b"], np.float32)
    w2w = np.asarray(inputs["W2_w"], np.float32)
    w2b = np.asarray(inputs["W2_b"], np.float32)
    vw = np.asarray(inputs["V_w"], np.float32)

    w1t = np.ascontiguousarray(w1w.T)
    w2t = np.ascontiguousarray(w2w.T)
    vby = np.zeros((128, 12), np.float32)
    vby[:, 0:4] = vw.reshape(4, 128).T
    vby[:, 4:8] = w1b.reshape(4, 128).T
    vby[:, 8:12] = w2b.reshape(4, 128).T
    ident = np.eye(128, dtype=np.float32)

    hid2 = hidden.reshape(BS, HID)
    in_maps = []
    for i in range(N_CORES):
        sl = slice(i * B, (i + 1) * B)
        in_maps.append(
            {
                "features": np.ascontiguousarray(features[sl]),
                "hidden": np.ascontiguousarray(hid2[sl]),
                "w1t": w1t,
                "w2t": w2t,
                "vby": vby,
                "ident": ident,
            }
        )
    return in_maps


def run(inputs, trace=False):
    nc = _get_nc()
    in_maps = _make_in_maps(inputs)
    res = run_bass_kernel_spmd(nc, in_maps, core_ids=list(range(N_CORES)), trace=trace)
    outs = [res.results[i]["out"] for i in range(N_CORES)]
    full = np.concatenate(outs, axis=0)
    ctx_v = np.ascontiguousarray(full[:, :EMB])
    attw = np.ascontiguousarray(full[:, EMB:]).reshape(BS, SEQ, 1)
    return (ctx_v, attw), res.exec_time_ns


def kernel(**inputs):
    (ctx_v, attw), _ = run(inputs, trace=False)
    return ctx_v, attw


# revision 29
# speedup vs baseline: 4.1045x; 4.1045x over previous
"""Trainium2 Bass kernel (raw bass, hand-pipelined) for additive attention.

Math per batch b (1024 batches, 128 per core, pure data parallel, 8 cores):
    f   = features[b] @ W1_w.T               # [seq=360, units=512]
    r   = relu(hidden[b] @ W2_w.T + W2_b)    # [units]
    a   = tanh(relu(f + W1_b) + r) = max(tanh(f + W1_b + r), tanh(r))
    s   = a @ V_w.T        (V_b dropped: softmax is shift-invariant)
    w   = softmax(s); ctx = w @ features[b]
returns (ctx [1024, 512] f32, w [1024, 360, 1] f32)

Raw-bass engine streams with manual semaphores (the bundled Tile scheduler
emits multi-wait instructions this walrus rejects). Steady-state schedule,
one "iteration" i per batch (groups g of 4 batches; q = g%2 slot):

  SP  iter i: outputs(g=(i-12)/4) | loads(i+2) | xbar transposes(i)
  DVE iter i: cast(i) | ps-memset((i-1)/4) | A'(i-2) | preadd(i-2)
              | softmaxA((i-6)/4) | softmaxB+wt-copies((i-8)/4)
              | cs-copy((i-10)/4)
  ACT iter i: exp((i-7)/4) | tanh(i) x4
  PE  iter i: main-mm(i) | score(i-2) | wb-transposes((i-8)/4) | ctx((i-9)/4)

Main matmul: out[u,s] = W1T.T @ FT in bf16, N=360 (FT from DMA-xbar
transpose of the DVE-cast bf16 features). Score reduce via ones-vector
matmul into per-batch PSUM partitions 32j (col-group packed, 4 batches /
bank). Context matmuls 4-way col-packed against the bf16 natural features.
"""

import sys
import numpy as np

sys.path.insert(0, "/opt/trn_rl_repo")

import concourse.bass as bass  # noqa: E402
from concourse import mybir  # noqa: E402
from concourse.bass_utils import run_bass_kernel_spmd  # noqa: E402

F32 = mybir.dt.float32
BF16 = mybir.dt.bfloat16
ALU = mybir.AluOpType
ACTF = mybir.ActivationFunctionType
AX = mybir.AxisListType

N_CORES = 8
BS = 1024
B = BS // N_CORES  # 128
SEQ = 360
EMB = 512
HID = 512
UNITS = 512
G = B // 4  # 32 groups

S_CHUNKS = [(0, 128, 128), (128, 128, 128), (256, 104, 112)]  # (s0, rows, xbar_rows)
KC = 4
MC = 4

NB = 20   # bf16 feature slots
NT = 4    # transposed feature slots
NE = 6    # y/x/xs slots

OUT_W = EMB + SEQ
FTW = 368
VIT = 148  # virtual iterations (B + drain tail)


def build_nc():
    nc = bass.Bass(trn_type="TRN2", target_bir_lowering=False, debug=False)

    feat = nc.declare_dram_parameter("features", [B, SEQ, EMB], F32, isOutput=False).ap()
    hid = nc.declare_dram_parameter("hidden", [B, HID], F32, isOutput=False).ap()
    w1t = nc.declare_dram_parameter("w1t", [EMB, UNITS], F32, isOutput=False).ap()
    w2t = nc.declare_dram_parameter("w2t", [HID, UNITS], F32, isOutput=False).ap()
    vby = nc.declare_dram_parameter("vby", [128, 12], F32, isOutput=False).ap()
    ident = nc.declare_dram_parameter("ident", [128, 128], F32, isOutput=False).ap()
    out = nc.declare_dram_parameter("out", [B, OUT_W], F32, isOutput=True).ap()
    # vby cols: 0-3 V chunks, 4-7 W1_b chunks, 8-11 W2_b chunks

    A = lambda name, shape, dt=F32: nc.alloc_sbuf_tensor(name, shape, dt).ap()

    id_sb = A("id_sb", [128, 128])
    vby_sb = A("vby_sb", [128, 12])
    w1t_f = [A(f"w1tf{k}", [128, UNITS]) for k in range(KC)]
    w1t_b = [A(f"w1tb{k}", [128, UNITS], BF16) for k in range(KC)]
    w2t_f = [A(f"w2tf{k}", [128, UNITS]) for k in range(KC)]
    hid_sb = A("hid_sb", [128, HID])
    ht = [A(f"ht{k}", [128, 128]) for k in range(KC)]
    r_sb = [A(f"r{m}", [128, B]) for m in range(MC)]
    rb1 = [A(f"rb1{m}", [128, B]) for m in range(MC)]
    thr = [A(f"thr{m}", [128, B]) for m in range(MC)]
    ones_bf = A("ones_bf", [128, 1], BF16)

    fb = [A(f"fb{a}", [128, 3 * EMB], BF16) for a in range(NB)]
    ft = [A(f"ft{d}", [128, 12 * 128], BF16) for d in range(NT)]
    y = [[A(f"y{m}_{e}", [128, SEQ], BF16) for e in range(NE)] for m in range(MC)]
    x = [[A(f"x{m}_{e}", [128, SEQ], BF16) for e in range(NE)] for m in range(MC)]
    xs = [A(f"xs{e}", [128, SEQ], BF16) for e in range(NE)]
    mx = [A(f"mx{q}", [97, 1]) for q in range(4)]
    nmx = [A(f"nmx{q}", [97, 1]) for q in range(4)]
    se = [A(f"se{q}", [97, 1]) for q in range(4)]
    rs = [A(f"rs{q}", [97, 1]) for q in range(4)]
    eb = [A(f"eb{q}", [97, SEQ]) for q in range(4)]
    wb = [A(f"wb{q}", [97, SEQ]) for q in range(4)]
    wt = [A(f"wt{q}", [128, 3 * 97], BF16) for q in range(4)]
    cs = [A(f"cs{q}", [97, EMB]) for q in range(4)]

    P = lambda name, shape: nc.alloc_psum_tensor(name, shape, F32).ap()
    pf = [P(f"pf{m}", [128, 384]) for m in range(MC)]
    ps = [P(f"ps{q}", [97, SEQ]) for q in range(2)]
    pc = P("pc", [97, EMB])
    pt = P("pt", [128, 384])

    S = lambda name: nc.alloc_semaphore(name)
    s_prep = S("s_prep")
    s_ppe = S("s_ppe")
    s_ppv = S("s_ppv")
    s_phm = S("s_phm")
    s_pact = S("s_pact")
    s_prb = S("s_prb")
    s_load = S("s_load")
    s_tp = S("s_tp")
    s_mm = S("s_mm")
    s_tanh = S("s_tanh")
    s_sum = S("s_sum")
    s_pmz = S("s_pmz")
    s_sc = S("s_sc")
    s_max = S("s_max")
    s_exp = S("s_exp")
    s_soft = S("s_soft")
    s_wtp = S("s_wtp")
    s_wtc = S("s_wtc")
    s_ctx = S("s_ctx")
    s_csc = S("s_csc")
    s_wout = S("s_wout")

    N_PREP_DMAS = 11

    with nc.Block() as block:

        @block.sync
        def _(sync):
            sync.dma_start(out=id_sb, in_=ident).then_inc(s_prep, 16)
            sync.dma_start(out=vby_sb, in_=vby).then_inc(s_prep, 16)
            for k in range(KC):
                sync.dma_start(out=w1t_f[k], in_=w1t[k * 128:(k + 1) * 128, :]).then_inc(s_prep, 16)
                sync.dma_start(out=w2t_f[k], in_=w2t[k * 128:(k + 1) * 128, :]).then_inc(s_prep, 16)
            sync.dma_start(out=hid_sb, in_=hid).then_inc(s_prep, 16)

            for i in range(VIT):
                # xbar transpose for batch i (one 3D instruction, 12 blocks)
                if i < B:
                    sync.wait_ge(s_load, 32 * (i + 1))
                    if i >= NT:
                        sync.wait_ge(s_mm, 4 * (i - NT) + 4)
                    sync.dma_start(
                        out=ft[i % NT].rearrange("p (b w) -> p b w", b=12),
                        in_=fb[i % NB],
                        transpose=True,
                    ).then_inc(s_tp, 16)
                # outputs for group (i-18)//4 (strided-partition reads)
                if i % 4 == 2 and i >= 18 and (i - 18) // 4 < G:
                    go = (i - 18) // 4
                    sync.wait_ge(s_soft, go + 1)
                    sync.dma_start(
                        out=out[4 * go:4 * go + 4, EMB:EMB + SEQ],
                        in_=wb[go % 4][0:97:32, :],
                    ).then_inc(s_wout, 16)
                    sync.wait_ge(s_csc, go + 1)
                    sync.dma_start(
                        out=out[4 * go:4 * go + 4, 0:EMB],
                        in_=cs[go % 4][0:97:32, :],
                    ).then_inc(s_wout, 16)
            sync.wait_ge(s_wout, 32 * G)

        @block.gpsimd
        def _(gpsimd):
            for i in range(VIT):
                # cast-loads (f32 DRAM -> bf16 SBUF) for batch i
                if i < B:
                    if i >= 3:
                        gpsimd.wait_ge(s_tp, 16 * (i - 2))  # throttle lookahead
                    if i >= NB:
                        gpsimd.wait_ge(s_ctx, (i - NB) // 4 + 1)
                    gpsimd.dma_start(
                        out=fb[i % NB][:, 0:1024].rearrange("p (c e) -> p c e", c=2),
                        in_=feat[i, 0:256, :].rearrange("(c p) e -> p c e", p=128),
                    ).then_inc(s_load, 16)
                    gpsimd.dma_start(
                        out=fb[i % NB][:104, 1024:1536], in_=feat[i, 256:360, :]
                    ).then_inc(s_load, 16)

        @block.vector
        def _(vector):
            # prep: hT copies (ping-pong over pt with PE)
            for k in range(KC):
                vector.wait_ge(s_ppe, k + 1)
                vector.tensor_copy(ht[k], pt[:, :128]).then_inc(s_ppv, 1)
            for k in range(KC):
                vector.tensor_copy(w1t_b[k], w1t_f[k])
            vector.memset(ones_bf, 1.0)
            vector.memset(ps[0], 0.0).then_inc(s_pmz, 1)
            vector.memset(ps[1], 0.0).then_inc(s_pmz, 1)
            for m in range(MC):
                vector.wait_ge(s_pact, m + 1)
                vector.tensor_scalar_add(
                    rb1[m], r_sb[m], vby_sb[:, 4 + m:5 + m]
                ).then_inc(s_prb, 1)

            for i in range(VIT):
                # A'(i-3) + preadd(i-3)
                if 3 <= i < B + 3:
                    b = i - 3
                    for m in range(MC):
                        vector.wait_ge(s_tanh, 4 * b + m + 1)
                        vector.tensor_scalar(
                            out=x[m][b % NE],
                            in0=y[m][b % NE],
                            scalar1=thr[m][:, b:b + 1],
                            scalar2=vby_sb[:, m:m + 1],
                            op0=ALU.max,
                            op1=ALU.mult,
                        )
                    if b >= NE:
                        vector.wait_ge(s_sc, b - NE + 1)  # xs slot WAR
                    vector.tensor_add(xs[b % NE], x[0][b % NE], x[1][b % NE])
                    vector.tensor_add(xs[b % NE], xs[b % NE], x[2][b % NE])
                    vector.tensor_add(
                        xs[b % NE], xs[b % NE], x[3][b % NE]
                    ).then_inc(s_sum, 1)
                # softmax A for group (i-8)//4
                if i % 4 == 0 and i >= 8 and (i - 8) // 4 < G:
                    ga = (i - 8) // 4
                    vector.wait_ge(s_sc, 4 * ga + 4)
                    if ga >= 4:
                        vector.wait_ge(s_exp, ga - 3)  # mx/nmx slot WAR
                    vector.tensor_reduce(mx[ga % 4], ps[ga % 2], AX.X, ALU.max)
                    vector.drain()
                    vector.tensor_scalar_mul(nmx[ga % 4], mx[ga % 4], -1.0).then_inc(
                        s_max, 1
                    )
                # softmax B for group (i-10)//4
                if i % 4 == 2 and i >= 10 and (i - 10) // 4 < G:
                    gb = (i - 10) // 4
                    vector.wait_ge(s_exp, gb + 1)
                    if gb >= 4:
                        vector.wait_ge(s_wout, 32 * (gb - 3))  # wb slot WAR
                    vector.reciprocal(rs[gb % 4], se[gb % 4])
                    vector.drain()
                    vector.tensor_scalar_mul(
                        wb[gb % 4], eb[gb % 4], rs[gb % 4]
                    ).then_inc(s_soft, 1)


        @block.scalar
        def _(scalar):
            for m in range(MC):
                scalar.wait_ge(s_phm, m + 1)
                scalar.activation(
                    r_sb[m], pf[m][:, :B], ACTF.Relu,
                    bias=vby_sb[:, 8 + m:9 + m], scale=1.0,
                )
                scalar.activation(thr[m], r_sb[m], ACTF.Tanh).then_inc(s_pact, 1)

            for i in range(VIT):
                # wt copy for group (i-12)//4 (pt -> wt, cast to bf16)
                if i % 4 == 0 and i >= 12 and (i - 12) // 4 < G:
                    gb = (i - 12) // 4
                    scalar.wait_ge(s_wtp, gb + 1)
                    if gb >= 4:
                        scalar.wait_ge(s_ctx, gb - 3)  # wt slot WAR
                    scalar.activation(
                        wt[gb % 4], pt[:, :291], ACTF.Copy
                    ).then_inc(s_wtc, 1)
                # cs copy for group (i-14)//4
                if i % 4 == 2 and i >= 14 and (i - 14) // 4 < G:
                    gcs = (i - 14) // 4
                    scalar.wait_ge(s_ctx, gcs + 1)
                    if gcs >= 4:
                        scalar.wait_ge(s_wout, 32 * (gcs - 3))  # cs slot WAR
                    scalar.activation(cs[gcs % 4], pc, ACTF.Copy).then_inc(s_csc, 1)
                # exp for group (i-9)//4
                if i % 4 == 1 and i >= 9 and (i - 9) // 4 < G:
                    ge = (i - 9) // 4
                    scalar.wait_ge(s_max, ge + 1)
                    if ge >= 4:
                        scalar.wait_ge(s_soft, ge - 3)  # eb/se slot WAR
                    scalar.activation(
                        eb[ge % 4], ps[ge % 2], ACTF.Exp,
                        bias=nmx[ge % 4], scale=1.0, accum_out=se[ge % 4],
                    ).then_inc(s_exp, 1)
                # tanh(i-1)
                if 1 <= i < B + 1:
                    b = i - 1
                    if b == 0:
                        scalar.wait_ge(s_prb, 4)
                    if b >= NE:
                        scalar.wait_ge(s_sum, b - NE + 1)  # y slot WAR
                    for m in range(MC):
                        scalar.wait_ge(s_mm, 4 * b + m + 1)
                        scalar.activation(
                            y[m][b % NE], pf[m][:, :SEQ], ACTF.Tanh,
                            bias=rb1[m][:, b:b + 1], scale=1.0,
                        ).then_inc(s_tanh, 1)

        @block.tensor
        def _(tensor):
            tensor.wait_ge(s_prep, 16 * N_PREP_DMAS)
            for k in range(KC):
                if k > 0:
                    tensor.wait_ge(s_ppv, k)
                tensor.transpose(
                    pt[:, :128], hid_sb[:, k * 128:(k + 1) * 128], id_sb
                ).then_inc(s_ppe, 1)
            tensor.wait_ge(s_ppv, 4)
            for m in range(MC):
                for k in range(KC):
                    ins = tensor.matmul(
                        pf[m][:, :B],
                        lhsT=w2t_f[k][:, m * 128:(m + 1) * 128],
                        rhs=ht[k],
                        start=(k == 0),
                        stop=(k == KC - 1),
                    )
                    if k == KC - 1:
                        ins.then_inc(s_phm, 1)

            for i in range(VIT):
                # main matmul(i-1)
                if 1 <= i < B + 1:
                    b = i - 1
                    tensor.wait_ge(s_tp, 16 * (b + 1))
                    if b == 0:
                        tensor.wait_ge(s_prb, 1)  # w1t_b + ones done (DVE order)
                    if b >= 1:
                        tensor.wait_ge(s_tanh, 4 * b)
                    else:
                        tensor.wait_ge(s_pact, 4)
                    ft3 = ft[b % NT].rearrange("p (c w) -> p c w", c=3)
                    for m in range(MC):
                        for k in range(KC):
                            ins = tensor.matmul(
                                pf[m],
                                lhsT=w1t_b[k][:, m * 128:(m + 1) * 128],
                                rhs=ft3[:, :, k * 128:(k + 1) * 128],
                                start=(k == 0),
                                stop=(k == KC - 1),
                            )
                            if k == KC - 1:
                                ins.then_inc(s_mm, 1)
                # score(i-4)
                if 4 <= i < B + 4:
                    b = i - 4
                    g, j = b // 4, b % 4
                    tensor.wait_ge(s_sum, b + 1)
                    if g < 2:
                        tensor.wait_ge(s_pmz, 2)
                    else:
                        tensor.wait_ge(s_exp, g - 1)  # ps bank WAR (implies pmz)
                    tensor.matmul(
                        ps[g % 2][32 * j:32 * j + 1, :],
                        lhsT=ones_bf,
                        rhs=xs[b % NE],
                        start=True,
                        stop=True,
                        tile_position=(0, 32 * j),
                        skip_group_check=True,
                    ).then_inc(s_sc, 1)
                # wb transposes for group (i-11)//4
                if i % 4 == 3 and i >= 11 and (i - 11) // 4 < G:
                    gw = (i - 11) // 4
                    tensor.wait_ge(s_soft, gw + 1)
                    tensor.wait_ge(s_wtc, gw)  # pt WAR vs wt-copy(g-1)
                    for c, (s0, rows, _) in enumerate(S_CHUNKS):
                        ins = tensor.transpose(
                            pt[:rows, c * 97:c * 97 + 97],
                            wb[gw % 4][:, s0:s0 + rows],
                            id_sb[:97, :97],
                        )
                        if c == 2:
                            ins.then_inc(s_wtp, 1)
                # context matmuls for group (i-13)//4
                if i % 4 == 1 and i >= 13 and (i - 13) // 4 < G:
                    gc = (i - 13) // 4
                    tensor.wait_ge(s_wtc, gc + 1)
                    if gc >= 1:
                        tensor.wait_ge(s_csc, gc)  # pc WAR
                    for jj in range(4):
                        bg = 4 * gc + jj
                        for c, (s0, rows, _) in enumerate(S_CHUNKS):
                            ins = tensor.matmul(
                                pc[32 * jj:32 * jj + 1, :],
                                lhsT=wt[gc % 4][:rows, c * 97 + 32 * jj:c * 97 + 32 * jj + 1],
                                rhs=fb[bg % NB][:rows, c * EMB:(c + 1) * EMB],
                                start=(c == 0),
                                stop=(c == 2),
                                tile_position=(0, 32 * jj),
                                skip_group_check=True,
                            )
                            if jj == 3 and c == 2:
                                ins.then_inc(s_ctx, 1)

    return nc


_NC_CACHE = None


def _get_nc():
    global _NC_CACHE
    if _NC_CACHE is None:
        _NC_CACHE = build_nc()
    return _NC_CACHE


def _make_in_maps(inputs):
    hidden = np.asarray(inputs["hidden"], np.float32)
    features = np.asarray(inputs["features"], np.float32)
    w1w = np.asarray(inputs["W1_w"], np.float32)
    w1b = np.asarray(inputs["W1_b"], np.float32)
    w2w = np.asarray(inputs["W2_w"], np.float32)
    w2b = np.asarray(inputs["W2_b"], np.float32)
    vw = np.asarray(inputs["V_w"], np.float32)

    w1t = np.ascontiguousarray(w1w.T)
    w2t = np.ascontiguousarray(w2w.T)
    vby = np.zeros((128, 12), np.float32)
    vby[:, 0:4] = vw.reshape(4, 128).T
    vby[:, 4:8] = w1b.reshape(4, 128).T
    vby[:, 8:12] = w2b.reshape(4, 128).T
    ident = np.eye(128, dtype=np.float32)

    hid2 = hidden.reshape(BS, HID)
    in_maps = []
    for i in range(N_CORES):
        sl = slice(i * B, (i + 1) * B)
        in_maps.append(
            {
                "features": np.ascontiguousarray(features[sl]),
                "hidden": np.ascontiguousarray(hid2[sl]),
                "w1t": w1t,
                "w2t": w2t,
                "vby": vby,
                "ident": ident,
            }
        )
    return in_maps


def run(inputs, trace=False):
    nc = _get_nc()
    in_maps = _make_in_maps(inputs)
    res = run_bass_kernel_spmd(nc, in_maps, core_ids=list(range(N_CORES)), trace=trace)
    outs = [res.results[i]["out"] for i in range(N_CORES)]
    full = np.concatenate(outs, axis=0)
    ctx_v = np.ascontiguousarray(full[:, :EMB])
    attw = np.ascontiguousarray(full[:, EMB:]).reshape(BS, SEQ, 1)
    return (ctx_v, attw), res.exec_time_ns


def kernel(**inputs):
    (ctx_v, attw), _ = run(inputs, trace=False)
    return ctx_v, attw


# revision 30
# speedup vs baseline: 4.1368x; 1.0079x over previous
"""Trainium2 Bass kernel (raw bass, hand-pipelined) for additive attention.

Math per batch b (1024 batches, 128 per core, pure data parallel, 8 cores):
    f   = features[b] @ W1_w.T               # [seq=360, units=512]
    r   = relu(hidden[b] @ W2_w.T + W2_b)    # [units]
    a   = tanh(relu(f + W1_b) + r) = max(tanh(f + W1_b + r), tanh(r))
    s   = a @ V_w.T        (V_b dropped: softmax is shift-invariant)
    w   = softmax(s); ctx = w @ features[b]
returns (ctx [1024, 512] f32, w [1024, 360, 1] f32)

Raw-bass engine streams with manual semaphores (the bundled Tile scheduler
emits multi-wait instructions this walrus rejects). Steady-state schedule,
one "iteration" i per batch (groups g of 4 batches; q = g%2 slot):

  SP  iter i: outputs(g=(i-12)/4) | loads(i+2) | xbar transposes(i)
  DVE iter i: cast(i) | ps-memset((i-1)/4) | A'(i-2) | preadd(i-2)
              | softmaxA((i-6)/4) | softmaxB+wt-copies((i-8)/4)
              | cs-copy((i-10)/4)
  ACT iter i: exp((i-7)/4) | tanh(i) x4
  PE  iter i: main-mm(i) | score(i-2) | wb-transposes((i-8)/4) | ctx((i-9)/4)

Main matmul: out[u,s] = W1T.T @ FT in bf16, N=360 (FT from DMA-xbar
transpose of the DVE-cast bf16 features). Score reduce via ones-vector
matmul into per-batch PSUM partitions 32j (col-group packed, 4 batches /
bank). Context matmuls 4-way col-packed against the bf16 natural features.
"""

import sys
import numpy as np

sys.path.insert(0, "/opt/trn_rl_repo")

import concourse.bass as bass  # noqa: E402
from concourse import mybir  # noqa: E402
from concourse.bass_utils import run_bass_kernel_spmd  # noqa: E402

F32 = mybir.dt.float32
BF16 = mybir.dt.bfloat16
ALU = mybir.AluOpType
ACTF = mybir.ActivationFunctionType
AX = mybir.AxisListType

N_CORES = 8
BS = 1024
B = BS // N_CORES  # 128
SEQ = 360
EMB = 512
HID = 512
UNITS = 512
G = B // 4  # 32 groups

S_CHUNKS = [(0, 128, 128), (128, 128, 128), (256, 104, 112)]  # (s0, rows, xbar_rows)
KC = 4
MC = 4

NB = 20   # bf16 feature slots
NT = 4    # transposed feature slots
NE = 6    # y/x/xs slots

OUT_W = EMB + SEQ
FTW = 368
VIT = 148  # virtual iterations (B + drain tail)


def build_nc():
    nc = bass.Bass(trn_type="TRN2", target_bir_lowering=False, debug=False)

    feat = nc.declare_dram_parameter("features", [B, SEQ, EMB], F32, isOutput=False).ap()
    hid = nc.declare_dram_parameter("hidden", [B, HID], F32, isOutput=False).ap()
    w1t = nc.declare_dram_parameter("w1t", [EMB, UNITS], F32, isOutput=False).ap()
    w2t = nc.declare_dram_parameter("w2t", [HID, UNITS], F32, isOutput=False).ap()
    vby = nc.declare_dram_parameter("vby", [128, 12], F32, isOutput=False).ap()
    ident = nc.declare_dram_parameter("ident", [128, 128], F32, isOutput=False).ap()
    out = nc.declare_dram_parameter("out", [B, OUT_W], F32, isOutput=True).ap()
    # vby cols: 0-3 V chunks, 4-7 W1_b chunks, 8-11 W2_b chunks

    A = lambda name, shape, dt=F32: nc.alloc_sbuf_tensor(name, shape, dt).ap()

    id_sb = A("id_sb", [128, 128])
    vby_sb = A("vby_sb", [128, 12])
    w1t_f = [A(f"w1tf{k}", [128, UNITS]) for k in range(KC)]
    w1t_b = [A(f"w1tb{k}", [128, UNITS], BF16) for k in range(KC)]
    w2t_f = [A(f"w2tf{k}", [128, UNITS]) for k in range(KC)]
    hid_sb = A("hid_sb", [128, HID])
    ht = [A(f"ht{k}", [128, 128]) for k in range(KC)]
    r_sb = [A(f"r{m}", [128, B]) for m in range(MC)]
    rb1 = [A(f"rb1{m}", [128, B]) for m in range(MC)]
    thr = [A(f"thr{m}", [128, B]) for m in range(MC)]
    ones_bf = A("ones_bf", [128, 1], BF16)

    fb = [A(f"fb{a}", [128, 3 * EMB], BF16) for a in range(NB)]
    ft = [A(f"ft{d}", [128, 12 * 128], BF16) for d in range(NT)]
    y = [[A(f"y{m}_{e}", [128, SEQ], BF16) for e in range(NE)] for m in range(MC)]
    x = [[A(f"x{m}_{e}", [128, SEQ], BF16) for e in range(NE)] for m in range(MC)]
    xs = [A(f"xs{e}", [128, SEQ], BF16) for e in range(NE)]
    mx = [A(f"mx{q}", [97, 1]) for q in range(4)]
    nmx = [A(f"nmx{q}", [97, 1]) for q in range(4)]
    se = [A(f"se{q}", [97, 1]) for q in range(4)]
    rs = [A(f"rs{q}", [97, 1]) for q in range(4)]
    eb = [A(f"eb{q}", [97, SEQ]) for q in range(4)]
    wb = [A(f"wb{q}", [97, SEQ]) for q in range(4)]
    wt = [A(f"wt{q}", [128, 3 * 97], BF16) for q in range(4)]
    cs = [A(f"cs{q}", [97, EMB]) for q in range(4)]

    P = lambda name, shape: nc.alloc_psum_tensor(name, shape, F32).ap()
    pf = [P(f"pf{m}", [128, 384]) for m in range(MC)]
    ps = [P(f"ps{q}", [97, SEQ]) for q in range(2)]
    pc = P("pc", [97, EMB])
    pt = P("pt", [128, 384])

    S = lambda name: nc.alloc_semaphore(name)
    s_prep = S("s_prep")
    s_ppe = S("s_ppe")
    s_ppv = S("s_ppv")
    s_phm = S("s_phm")
    s_pact = S("s_pact")
    s_prb = S("s_prb")
    s_load = S("s_load")
    s_tp = S("s_tp")
    s_mm = S("s_mm")
    s_tanh = S("s_tanh")
    s_sum = S("s_sum")
    s_pmz = S("s_pmz")
    s_sc = S("s_sc")
    s_max = S("s_max")
    s_exp = S("s_exp")
    s_soft = S("s_soft")
    s_wtp = S("s_wtp")
    s_wtc = S("s_wtc")
    s_ctx = S("s_ctx")
    s_csc = S("s_csc")
    s_wout = S("s_wout")

    N_PREP_DMAS = 11

    with nc.Block() as block:

        @block.sync
        def _(sync):
            sync.dma_start(out=id_sb, in_=ident).then_inc(s_prep, 16)
            sync.dma_start(out=vby_sb, in_=vby).then_inc(s_prep, 16)
            for k in range(KC):
                sync.dma_start(out=w1t_f[k], in_=w1t[k * 128:(k + 1) * 128, :]).then_inc(s_prep, 16)
                sync.dma_start(out=w2t_f[k], in_=w2t[k * 128:(k + 1) * 128, :]).then_inc(s_prep, 16)
            sync.dma_start(out=hid_sb, in_=hid).then_inc(s_prep, 16)

            for i in range(VIT):
                # xbar transpose for batch i (one 3D instruction, 12 blocks)
                if i < B:
                    sync.wait_ge(s_load, 32 * (i + 1))
                    if i >= NT:
                        sync.wait_ge(s_mm, 4 * (i - NT) + 4)
                    sync.dma_start(
                        out=ft[i % NT].rearrange("p (b w) -> p b w", b=12),
                        in_=fb[i % NB],
                        transpose=True,
                    ).then_inc(s_tp, 16)
                # outputs for group (i-18)//4 (strided-partition reads)
                if i % 4 == 2 and i >= 18 and (i - 18) // 4 < G:
                    go = (i - 18) // 4
                    sync.wait_ge(s_soft, go + 1)
                    sync.dma_start(
                        out=out[4 * go:4 * go + 4, EMB:EMB + SEQ],
                        in_=wb[go % 4][0:97:32, :],
                    ).then_inc(s_wout, 16)
                    sync.wait_ge(s_csc, go + 1)
                    sync.dma_start(
                        out=out[4 * go:4 * go + 4, 0:EMB],
                        in_=cs[go % 4][0:97:32, :],
                    ).then_inc(s_wout, 16)
            sync.wait_ge(s_wout, 32 * G)

        @block.gpsimd
        def _(gpsimd):
            for i in range(VIT):
                # cast-loads (f32 DRAM -> bf16 SBUF) for batch i
                if i < B:
                    if i >= 3:
                        gpsimd.wait_ge(s_tp, 16 * (i - 2))  # throttle lookahead
                    if i >= NB:
                        gpsimd.wait_ge(s_ctx, (i - NB) // 4 + 1)
                    gpsimd.dma_start(
                        out=fb[i % NB][:, 0:1024].rearrange("p (c e) -> p c e", c=2),
                        in_=feat[i, 0:256, :].rearrange("(c p) e -> p c e", p=128),
                    ).then_inc(s_load, 16)
                    gpsimd.dma_start(
                        out=fb[i % NB][:104, 1024:1536], in_=feat[i, 256:360, :]
                    ).then_inc(s_load, 16)

        @block.vector
        def _(vector):
            # prep: hT copies (ping-pong over pt with PE)
            for k in range(KC):
                vector.wait_ge(s_ppe, k + 1)
                vector.tensor_copy(ht[k], pt[:, :128]).then_inc(s_ppv, 1)
            for k in range(KC):
                vector.tensor_copy(w1t_b[k], w1t_f[k])
            vector.memset(ones_bf, 1.0)
            vector.memset(ps[0], 0.0).then_inc(s_pmz, 1)
            vector.memset(ps[1], 0.0).then_inc(s_pmz, 1)
            for m in range(MC):
                vector.wait_ge(s_pact, m + 1)
                vector.tensor_scalar_add(
                    rb1[m], r_sb[m], vby_sb[:, 4 + m:5 + m]
                ).then_inc(s_prb, 1)

            for i in range(VIT):
                # A'(i-3) + preadd(i-3)
                if 3 <= i < B + 3:
                    b = i - 3
                    for m in range(MC):
                        vector.wait_ge(s_tanh, 4 * b + m + 1)
                        vector.tensor_scalar(
                            out=x[m][b % NE],
                            in0=y[m][b % NE],
                            scalar1=thr[m][:, b:b + 1],
                            scalar2=vby_sb[:, m:m + 1],
                            op0=ALU.max,
                            op1=ALU.mult,
                        )
                    if b >= NE:
                        vector.wait_ge(s_sc, b - NE + 1)  # xs slot WAR
                    vector.tensor_add(xs[b % NE], x[0][b % NE], x[1][b % NE])
                    vector.tensor_add(xs[b % NE], xs[b % NE], x[2][b % NE])
                    vector.tensor_add(
                        xs[b % NE], xs[b % NE], x[3][b % NE]
                    ).then_inc(s_sum, 1)
                # softmax A for group (i-8)//4
                if i % 4 == 0 and i >= 8 and (i - 8) // 4 < G:
                    ga = (i - 8) // 4
                    vector.wait_ge(s_sc, 4 * ga + 4)
                    if ga >= 4:
                        vector.wait_ge(s_exp, ga - 3)  # mx/nmx slot WAR
                    vector.tensor_reduce(mx[ga % 4], ps[ga % 2], AX.X, ALU.max)
                    vector.drain()
                    vector.tensor_scalar_mul(nmx[ga % 4], mx[ga % 4], -1.0).then_inc(
                        s_max, 1
                    )
                # softmax B for group (i-10)//4
                if i % 4 == 2 and i >= 10 and (i - 10) // 4 < G:
                    gb = (i - 10) // 4
                    vector.wait_ge(s_exp, gb + 1)
                    if gb >= 4:
                        vector.wait_ge(s_wout, 32 * (gb - 3))  # wb slot WAR
                    vector.reciprocal(rs[gb % 4], se[gb % 4])
                    vector.drain()
                    vector.tensor_scalar_mul(
                        wb[gb % 4], eb[gb % 4], rs[gb % 4]
                    ).then_inc(s_soft, 1)


        @block.scalar
        def _(scalar):
            for m in range(MC):
                scalar.wait_ge(s_phm, m + 1)
                scalar.activation(
                    r_sb[m], pf[m][:, :B], ACTF.Relu,
                    bias=vby_sb[:, 8 + m:9 + m], scale=1.0,
                )
                scalar.activation(thr[m], r_sb[m], ACTF.Tanh).then_inc(s_pact, 1)

            for i in range(VIT):
                # wt copy for group (i-12)//4 (pt -> wt, cast to bf16)
                if i % 4 == 0 and i >= 12 and (i - 12) // 4 < G:
                    gb = (i - 12) // 4
                    scalar.wait_ge(s_wtp, gb + 1)
                    if gb >= 4:
                        scalar.wait_ge(s_ctx, gb - 3)  # wt slot WAR
                    scalar.activation(
                        wt[gb % 4], pt[:, :291], ACTF.Copy
                    ).then_inc(s_wtc, 1)
                # cs copy for group (i-14)//4
                if i % 4 == 2 and i >= 14 and (i - 14) // 4 < G:
                    gcs = (i - 14) // 4
                    scalar.wait_ge(s_ctx, gcs + 1)
                    if gcs >= 4:
                        scalar.wait_ge(s_wout, 32 * (gcs - 3))  # cs slot WAR
                    scalar.activation(cs[gcs % 4], pc, ACTF.Copy).then_inc(s_csc, 1)
                # exp for group (i-9)//4
                if i % 4 == 1 and i >= 9 and (i - 9) // 4 < G:
                    ge = (i - 9) // 4
                    scalar.wait_ge(s_max, ge + 1)
                    if ge >= 4:
                        scalar.wait_ge(s_soft, ge - 3)  # eb/se slot WAR
                    scalar.activation(
                        eb[ge % 4], ps[ge % 2], ACTF.Exp,
                        bias=nmx[ge % 4], scale=1.0, accum_out=se[ge % 4],
                    ).then_inc(s_exp, 1)
                # tanh(i-1)
                if 1 <= i < B + 1:
                    b = i - 1
                    if b == 0:
                        scalar.wait_ge(s_prb, 4)
                    if b >= NE:
                        scalar.wait_ge(s_sum, b - NE + 1)  # y slot WAR
                    for m in range(MC):
                        scalar.wait_ge(s_mm, 4 * b + m + 1)
                        scalar.activation(
                            y[m][b % NE], pf[m][:, :SEQ], ACTF.Tanh,
                            bias=rb1[m][:, b:b + 1], scale=1.0,
                        ).then_inc(s_tanh, 1)

        @block.tensor
        def _(tensor):
            tensor.wait_ge(s_prep, 16 * N_PREP_DMAS)
            for k in range(KC):
                if k > 0:
                    tensor.wait_ge(s_ppv, k)
                tensor.transpose(
                    pt[:, :128], hid_sb[:, k * 128:(k + 1) * 128], id_sb
                ).then_inc(s_ppe, 1)
            tensor.wait_ge(s_ppv, 4)
            for m in range(MC):
                for k in range(KC):
                    ins = tensor.matmul(
                        pf[m][:, :B],
                        lhsT=w2t_f[k][:, m * 128:(m + 1) * 128],
                        rhs=ht[k],
                        start=(k == 0),
                        stop=(k == KC - 1),
                    )
                    if k == KC - 1:
                        ins.then_inc(s_phm, 1)

            for i in range(VIT):
                # main matmul(i-1)
                if 1 <= i < B + 1:
                    b = i - 1
                    tensor.wait_ge(s_tp, 16 * (b + 1))
                    if b == 0:
                        tensor.wait_ge(s_prb, 1)  # w1t_b + ones done (DVE order)
                    for m in range(MC):
                        if b >= 1:
                            tensor.wait_ge(s_tanh, 4 * (b - 1) + m + 1)
                        else:
                            tensor.wait_ge(s_pact, m + 1)
                        ft3 = ft[b % NT].rearrange("p (c w) -> p c w", c=3)
                        for k in range(KC):
                            ins = tensor.matmul(
                                pf[m],
                                lhsT=w1t_b[k][:, m * 128:(m + 1) * 128],
                                rhs=ft3[:, :, k * 128:(k + 1) * 128],
                                start=(k == 0),
                                stop=(k == KC - 1),
                            )
                            if k == KC - 1:
                                ins.then_inc(s_mm, 1)
                # score(i-4)
                if 4 <= i < B + 4:
                    b = i - 4
                    g, j = b // 4, b % 4
                    tensor.wait_ge(s_sum, b + 1)
                    tensor.wait_ge(s_pmz, 2)
                    if g >= 2:
                        tensor.wait_ge(s_exp, g - 1)  # ps bank WAR
                    tensor.matmul(
                        ps[g % 2][32 * j:32 * j + 1, :],
                        lhsT=ones_bf,
                        rhs=xs[b % NE],
                        start=True,
                        stop=True,
                        tile_position=(0, 32 * j),
                        skip_group_check=True,
                    ).then_inc(s_sc, 1)
                # wb transposes for group (i-11)//4
                if i % 4 == 3 and i >= 11 and (i - 11) // 4 < G:
                    gw = (i - 11) // 4
                    tensor.wait_ge(s_soft, gw + 1)
                    tensor.wait_ge(s_wtc, gw)  # pt WAR vs wt-copy(g-1)
                    for c, (s0, rows, _) in enumerate(S_CHUNKS):
                        ins = tensor.transpose(
                            pt[:rows, c * 97:c * 97 + 97],
                            wb[gw % 4][:, s0:s0 + rows],
                            id_sb[:97, :97],
                        )
                        if c == 2:
                            ins.then_inc(s_wtp, 1)
                # context matmuls for group (i-13)//4
                if i % 4 == 1 and i >= 13 and (i - 13) // 4 < G:
                    gc = (i - 13) // 4
                    tensor.wait_ge(s_wtc, gc + 1)
                    if gc >= 1:
                        tensor.wait_ge(s_csc, gc)  # pc WAR
                    for jj in range(4):
                        bg = 4 * gc + jj
                        for c, (s0, rows, _) in enumerate(S_CHUNKS):
                            ins = tensor.matmul(
                                pc[32 * jj:32 * jj + 1, :],
                                lhsT=wt[gc % 4][:rows, c * 97 + 32 * jj:c * 97 + 32 * jj + 1],
                                rhs=fb[bg % NB][:rows, c * EMB:(c + 1) * EMB],
                                start=(c == 0),
                                stop=(c == 2),
                                tile_position=(0, 32 * jj),
                                skip_group_check=True,
                            )
                            if jj == 3 and c == 2:
                                ins.then_inc(s_ctx, 1)

    return nc


_NC_CACHE = None


def _get_nc():
    global _NC_CACHE
    if _NC_CACHE is None:
        _NC_CACHE = build_nc()
    return _NC_CACHE


def _make_in_maps(inputs):
    hidden = np.asarray(inputs["hidden"], np.float32)
    features = np.asarray(inputs["features"], np.float32)
    w1w = np.asarray(inputs["W1_w"], np.float32)
    w1b = np.asarray(inputs["W1_b"], np.float32)
    w2w = np.asarray(inputs["W2_w"], np.float32)
    w2b = np.asarray(inputs["W2_b"], np.float32)
    vw = np.asarray(inputs["V_w"], np.float32)

    w1t = np.ascontiguousarray(w1w.T)
    w2t = np.ascontiguousarray(w2w.T)
    vby = np.zeros((128, 12), np.float32)
    vby[:, 0:4] = vw.reshape(4, 128).T
    vby[:, 4:8] = w1b.reshape(4, 128).T
    vby[:, 8:12] = w2b.reshape(4, 128).T
    ident = np.eye(128, dtype=np.float32)

    hid2 = hidden.reshape(BS, HID)
    in_maps = []
    for i in range(N_CORES):
        sl = slice(i * B, (i + 1) * B)
        in_maps.append(
            {
                "features": np.ascontiguousarray(features[sl]),
                "hidden": np.ascontiguousarray(hid2[sl]),
                "w1t": w1t,
                "w2t": w2t,
                "vby": vby,
                "ident": ident,
            }
        )
    return in_maps


def run(inputs, trace=False):
    nc = _get_nc()
    in_maps = _make_in_maps(inputs)
    res = run_bass_kernel_spmd(nc, in_maps, core_ids=list(range(N_CORES)), trace=trace)
    outs = [res.results[i]["out"] for i in range(N_CORES)]
    full = np.concatenate(outs, axis=0)
    ctx_v = np.ascontiguousarray(full[:, :EMB])
    attw = np.ascontiguousarray(full[:, EMB:]).reshape(BS, SEQ, 1)
    return (ctx_v, attw), res.exec_time_ns


def kernel(**inputs):
    (ctx_v, attw), _ = run(inputs, trace=False)
    return ctx_v, attw


# revision 31
# speedup vs baseline: 4.1880x; 1.0124x over previous
"""Trainium2 Bass kernel (raw bass, hand-pipelined) for additive attention.

Math per batch b (1024 batches, 128 per core, pure data parallel, 8 cores):
    f   = features[b] @ W1_w.T               # [seq=360, units=512]
    r   = relu(hidden[b] @ W2_w.T + W2_b)    # [units]
    a   = tanh(relu(f + W1_b) + r) = max(tanh(f + W1_b + r), tanh(r))
    s   = a @ V_w.T        (V_b dropped: softmax is shift-invariant)
    w   = softmax(s); ctx = w @ features[b]
returns (ctx [1024, 512] f32, w [1024, 360, 1] f32)

Raw-bass engine streams with manual semaphores (the bundled Tile scheduler
emits multi-wait instructions this walrus rejects). Steady-state schedule,
one "iteration" i per batch (groups g of 4 batches; q = g%2 slot):

  SP  iter i: outputs(g=(i-12)/4) | loads(i+2) | xbar transposes(i)
  DVE iter i: cast(i) | ps-memset((i-1)/4) | A'(i-2) | preadd(i-2)
              | softmaxA((i-6)/4) | softmaxB+wt-copies((i-8)/4)
              | cs-copy((i-10)/4)
  ACT iter i: exp((i-7)/4) | tanh(i) x4
  PE  iter i: main-mm(i) | score(i-2) | wb-transposes((i-8)/4) | ctx((i-9)/4)

Main matmul: out[u,s] = W1T.T @ FT in bf16, N=360 (FT from DMA-xbar
transpose of the DVE-cast bf16 features). Score reduce via ones-vector
matmul into per-batch PSUM partitions 32j (col-group packed, 4 batches /
bank). Context matmuls 4-way col-packed against the bf16 natural features.
"""

import sys
import numpy as np

sys.path.insert(0, "/opt/trn_rl_repo")

import concourse.bass as bass  # noqa: E402
from concourse import mybir  # noqa: E402
from concourse.bass_utils import run_bass_kernel_spmd  # noqa: E402

F32 = mybir.dt.float32
BF16 = mybir.dt.bfloat16
ALU = mybir.AluOpType
ACTF = mybir.ActivationFunctionType
AX = mybir.AxisListType

N_CORES = 8
BS = 1024
B = BS // N_CORES  # 128
SEQ = 360
EMB = 512
HID = 512
UNITS = 512
G = B // 4  # 32 groups

S_CHUNKS = [(0, 128, 128), (128, 128, 128), (256, 104, 112)]  # (s0, rows, xbar_rows)
KC = 4
MC = 4

NB = 20   # bf16 feature slots
NT = 6    # transposed feature slots
NE = 6    # y/x/xs slots

OUT_W = EMB + SEQ
FTW = 368
VIT = 148  # virtual iterations (B + drain tail)


def build_nc():
    nc = bass.Bass(trn_type="TRN2", target_bir_lowering=False, debug=False)

    feat = nc.declare_dram_parameter("features", [B, SEQ, EMB], F32, isOutput=False).ap()
    hid = nc.declare_dram_parameter("hidden", [B, HID], F32, isOutput=False).ap()
    w1t = nc.declare_dram_parameter("w1t", [EMB, UNITS], F32, isOutput=False).ap()
    w2t = nc.declare_dram_parameter("w2t", [HID, UNITS], F32, isOutput=False).ap()
    vby = nc.declare_dram_parameter("vby", [128, 12], F32, isOutput=False).ap()
    ident = nc.declare_dram_parameter("ident", [128, 128], F32, isOutput=False).ap()
    out = nc.declare_dram_parameter("out", [B, OUT_W], F32, isOutput=True).ap()
    # vby cols: 0-3 V chunks, 4-7 W1_b chunks, 8-11 W2_b chunks

    A = lambda name, shape, dt=F32: nc.alloc_sbuf_tensor(name, shape, dt).ap()

    id_sb = A("id_sb", [128, 128])
    vby_sb = A("vby_sb", [128, 12])
    w1t_f = [A(f"w1tf{k}", [128, UNITS]) for k in range(KC)]
    w1t_b = [A(f"w1tb{k}", [128, UNITS], BF16) for k in range(KC)]
    w2t_f = [A(f"w2tf{k}", [128, UNITS]) for k in range(KC)]
    hid_sb = A("hid_sb", [128, HID])
    ht = [A(f"ht{k}", [128, 128]) for k in range(KC)]
    r_sb = [A(f"r{m}", [128, B]) for m in range(MC)]
    rb1 = [A(f"rb1{m}", [128, B]) for m in range(MC)]
    thr = [A(f"thr{m}", [128, B]) for m in range(MC)]
    ones_bf = A("ones_bf", [128, 1], BF16)

    fb = [A(f"fb{a}", [128, 3 * EMB], BF16) for a in range(NB)]
    ft = [A(f"ft{d}", [128, 12 * 128], BF16) for d in range(NT)]
    y = [[A(f"y{m}_{e}", [128, SEQ], BF16) for e in range(NE)] for m in range(MC)]
    x = [[A(f"x{m}_{e}", [128, SEQ], BF16) for e in range(NE)] for m in range(MC)]
    xs = [A(f"xs{e}", [128, SEQ], BF16) for e in range(NE)]
    mx = [A(f"mx{q}", [97, 1]) for q in range(4)]
    nmx = [A(f"nmx{q}", [97, 1]) for q in range(4)]
    se = [A(f"se{q}", [97, 1]) for q in range(4)]
    rs = [A(f"rs{q}", [97, 1]) for q in range(4)]
    eb = [A(f"eb{q}", [97, SEQ]) for q in range(4)]
    wb = [A(f"wb{q}", [97, SEQ]) for q in range(4)]
    wt = [A(f"wt{q}", [128, 3 * 97], BF16) for q in range(4)]
    cs = [A(f"cs{q}", [97, EMB]) for q in range(4)]

    P = lambda name, shape: nc.alloc_psum_tensor(name, shape, F32).ap()
    pf = [P(f"pf{m}", [128, 384]) for m in range(MC)]
    ps = [P(f"ps{q}", [97, SEQ]) for q in range(2)]
    pc = P("pc", [97, EMB])
    pt = P("pt", [128, 384])

    S = lambda name: nc.alloc_semaphore(name)
    s_prep = S("s_prep")
    s_ppe = S("s_ppe")
    s_ppv = S("s_ppv")
    s_phm = S("s_phm")
    s_pact = S("s_pact")
    s_prb = S("s_prb")
    s_load = S("s_load")
    s_tp = S("s_tp")
    s_mm = S("s_mm")
    s_tanh = S("s_tanh")
    s_sum = S("s_sum")
    s_pmz = S("s_pmz")
    s_sc = S("s_sc")
    s_max = S("s_max")
    s_exp = S("s_exp")
    s_soft = S("s_soft")
    s_wtp = S("s_wtp")
    s_wtc = S("s_wtc")
    s_ctx = S("s_ctx")
    s_csc = S("s_csc")
    s_wout = S("s_wout")

    N_PREP_DMAS = 11

    with nc.Block() as block:

        @block.sync
        def _(sync):
            sync.dma_start(out=id_sb, in_=ident).then_inc(s_prep, 16)
            sync.dma_start(out=vby_sb, in_=vby).then_inc(s_prep, 16)
            for k in range(KC):
                sync.dma_start(out=w1t_f[k], in_=w1t[k * 128:(k + 1) * 128, :]).then_inc(s_prep, 16)
                sync.dma_start(out=w2t_f[k], in_=w2t[k * 128:(k + 1) * 128, :]).then_inc(s_prep, 16)
            sync.dma_start(out=hid_sb, in_=hid).then_inc(s_prep, 16)

            for i in range(VIT):
                # xbar transpose for batch i (one 3D instruction, 12 blocks)
                if i < B:
                    sync.wait_ge(s_load, 32 * (i + 1))
                    if i >= NT:
                        sync.wait_ge(s_mm, 4 * (i - NT) + 4)
                    sync.dma_start(
                        out=ft[i % NT].rearrange("p (b w) -> p b w", b=12),
                        in_=fb[i % NB],
                        transpose=True,
                    ).then_inc(s_tp, 16)
                # outputs for group (i-18)//4 (strided-partition reads)
                if i % 4 == 2 and i >= 18 and (i - 18) // 4 < G:
                    go = (i - 18) // 4
                    sync.wait_ge(s_soft, go + 1)
                    sync.dma_start(
                        out=out[4 * go:4 * go + 4, EMB:EMB + SEQ],
                        in_=wb[go % 4][0:97:32, :],
                    ).then_inc(s_wout, 16)
                    sync.wait_ge(s_csc, go + 1)
                    sync.dma_start(
                        out=out[4 * go:4 * go + 4, 0:EMB],
                        in_=cs[go % 4][0:97:32, :],
                    ).then_inc(s_wout, 16)
            sync.wait_ge(s_wout, 32 * G)

        @block.gpsimd
        def _(gpsimd):
            for i in range(VIT):
                # cast-loads (f32 DRAM -> bf16 SBUF) for batch i
                if i < B:
                    if i >= 3:
                        gpsimd.wait_ge(s_tp, 16 * (i - 2))  # throttle lookahead
                    if i >= NB:
                        gpsimd.wait_ge(s_ctx, (i - NB) // 4 + 1)
                    gpsimd.dma_start(
                        out=fb[i % NB][:, 0:1024].rearrange("p (c e) -> p c e", c=2),
                        in_=feat[i, 0:256, :].rearrange("(c p) e -> p c e", p=128),
                    ).then_inc(s_load, 16)
                    gpsimd.dma_start(
                        out=fb[i % NB][:104, 1024:1536], in_=feat[i, 256:360, :]
                    ).then_inc(s_load, 16)

        @block.vector
        def _(vector):
            # prep: hT copies (ping-pong over pt with PE)
            for k in range(KC):
                vector.wait_ge(s_ppe, k + 1)
                vector.tensor_copy(ht[k], pt[:, :128]).then_inc(s_ppv, 1)
            for k in range(KC):
                vector.tensor_copy(w1t_b[k], w1t_f[k])
            vector.memset(ones_bf, 1.0)
            vector.memset(ps[0], 0.0).then_inc(s_pmz, 1)
            vector.memset(ps[1], 0.0).then_inc(s_pmz, 1)
            for m in range(MC):
                vector.wait_ge(s_pact, m + 1)
                vector.tensor_scalar_add(
                    rb1[m], r_sb[m], vby_sb[:, 4 + m:5 + m]
                ).then_inc(s_prb, 1)

            for i in range(VIT):
                # A'(i-3) + preadd(i-3)
                if 3 <= i < B + 3:
                    b = i - 3
                    for m in range(MC):
                        vector.wait_ge(s_tanh, 4 * b + m + 1)
                        vector.tensor_scalar(
                            out=x[m][b % NE],
                            in0=y[m][b % NE],
                            scalar1=thr[m][:, b:b + 1],
                            scalar2=vby_sb[:, m:m + 1],
                            op0=ALU.max,
                            op1=ALU.mult,
                        )
                    if b >= NE:
                        vector.wait_ge(s_sc, b - NE + 1)  # xs slot WAR
                    vector.tensor_add(xs[b % NE], x[0][b % NE], x[1][b % NE])
                    vector.tensor_add(xs[b % NE], xs[b % NE], x[2][b % NE])
                    vector.tensor_add(
                        xs[b % NE], xs[b % NE], x[3][b % NE]
                    ).then_inc(s_sum, 1)
                # softmax A for group (i-8)//4
                if i % 4 == 0 and i >= 8 and (i - 8) // 4 < G:
                    ga = (i - 8) // 4
                    vector.wait_ge(s_sc, 4 * ga + 4)
                    if ga >= 4:
                        vector.wait_ge(s_exp, ga - 3)  # mx/nmx slot WAR
                    vector.tensor_reduce(mx[ga % 4], ps[ga % 2], AX.X, ALU.max)
                    vector.drain()
                    vector.tensor_scalar_mul(nmx[ga % 4], mx[ga % 4], -1.0).then_inc(
                        s_max, 1
                    )
                # softmax B for group (i-10)//4
                if i % 4 == 2 and i >= 10 and (i - 10) // 4 < G:
                    gb = (i - 10) // 4
                    vector.wait_ge(s_exp, gb + 1)
                    if gb >= 4:
                        vector.wait_ge(s_wout, 32 * (gb - 3))  # wb slot WAR
                    vector.reciprocal(rs[gb % 4], se[gb % 4])
                    vector.drain()
                    vector.tensor_scalar_mul(
                        wb[gb % 4], eb[gb % 4], rs[gb % 4]
                    ).then_inc(s_soft, 1)


        @block.scalar
        def _(scalar):
            for m in range(MC):
                scalar.wait_ge(s_phm, m + 1)
                scalar.activation(
                    r_sb[m], pf[m][:, :B], ACTF.Relu,
                    bias=vby_sb[:, 8 + m:9 + m], scale=1.0,
                )
                scalar.activation(thr[m], r_sb[m], ACTF.Tanh).then_inc(s_pact, 1)

            for i in range(VIT):
                # wt copy for group (i-12)//4 (pt -> wt, cast to bf16)
                if i % 4 == 0 and i >= 12 and (i - 12) // 4 < G:
                    gb = (i - 12) // 4
                    scalar.wait_ge(s_wtp, gb + 1)
                    if gb >= 4:
                        scalar.wait_ge(s_ctx, gb - 3)  # wt slot WAR
                    scalar.activation(
                        wt[gb % 4], pt[:, :291], ACTF.Copy
                    ).then_inc(s_wtc, 1)
                # cs copy for group (i-14)//4
                if i % 4 == 2 and i >= 14 and (i - 14) // 4 < G:
                    gcs = (i - 14) // 4
                    scalar.wait_ge(s_ctx, gcs + 1)
                    if gcs >= 4:
                        scalar.wait_ge(s_wout, 32 * (gcs - 3))  # cs slot WAR
                    scalar.activation(cs[gcs % 4], pc, ACTF.Copy).then_inc(s_csc, 1)
                # exp for group (i-9)//4
                if i % 4 == 1 and i >= 9 and (i - 9) // 4 < G:
                    ge = (i - 9) // 4
                    scalar.wait_ge(s_max, ge + 1)
                    if ge >= 4:
                        scalar.wait_ge(s_soft, ge - 3)  # eb/se slot WAR
                    scalar.activation(
                        eb[ge % 4], ps[ge % 2], ACTF.Exp,
                        bias=nmx[ge % 4], scale=1.0, accum_out=se[ge % 4],
                    ).then_inc(s_exp, 1)
                # tanh(i-1)
                if 1 <= i < B + 1:
                    b = i - 1
                    if b == 0:
                        scalar.wait_ge(s_prb, 4)
                    if b >= NE:
                        scalar.wait_ge(s_sum, b - NE + 1)  # y slot WAR
                    for m in range(MC):
                        scalar.wait_ge(s_mm, 4 * b + m + 1)
                        scalar.activation(
                            y[m][b % NE], pf[m][:, :SEQ], ACTF.Tanh,
                            bias=rb1[m][:, b:b + 1], scale=1.0,
                        ).then_inc(s_tanh, 1)

        @block.tensor
        def _(tensor):
            tensor.wait_ge(s_prep, 16 * N_PREP_DMAS)
            for k in range(KC):
                if k > 0:
                    tensor.wait_ge(s_ppv, k)
                tensor.transpose(
                    pt[:, :128], hid_sb[:, k * 128:(k + 1) * 128], id_sb
                ).then_inc(s_ppe, 1)
            tensor.wait_ge(s_ppv, 4)
            for m in range(MC):
                for k in range(KC):
                    ins = tensor.matmul(
                        pf[m][:, :B],
                        lhsT=w2t_f[k][:, m * 128:(m + 1) * 128],
                        rhs=ht[k],
                        start=(k == 0),
                        stop=(k == KC - 1),
                    )
                    if k == KC - 1:
                        ins.then_inc(s_phm, 1)

            for i in range(VIT):
                # main matmul(i-1)
                if 1 <= i < B + 1:
                    b = i - 1
                    tensor.wait_ge(s_tp, 16 * (b + 1))
                    if b == 0:
                        tensor.wait_ge(s_prb, 1)  # w1t_b + ones done (DVE order)
                    for m in range(MC):
                        if b >= 1:
                            tensor.wait_ge(s_tanh, 4 * (b - 1) + m + 1)
                        else:
                            tensor.wait_ge(s_pact, m + 1)
                        ft3 = ft[b % NT].rearrange("p (c w) -> p c w", c=3)
                        for k in range(KC):
                            ins = tensor.matmul(
                                pf[m],
                                lhsT=w1t_b[k][:, m * 128:(m + 1) * 128],
                                rhs=ft3[:, :, k * 128:(k + 1) * 128],
                                start=(k == 0),
                                stop=(k == KC - 1),
                            )
                            if k == KC - 1:
                                ins.then_inc(s_mm, 1)
                # score(i-4)
                if 4 <= i < B + 4:
                    b = i - 4
                    g, j = b // 4, b % 4
                    tensor.wait_ge(s_sum, b + 1)
                    tensor.wait_ge(s_pmz, 2)
                    if g >= 2:
                        tensor.wait_ge(s_exp, g - 1)  # ps bank WAR
                    tensor.matmul(
                        ps[g % 2][32 * j:32 * j + 1, :],
                        lhsT=ones_bf,
                        rhs=xs[b % NE],
                        start=True,
                        stop=True,
                        tile_position=(0, 32 * j),
                        skip_group_check=True,
                    ).then_inc(s_sc, 1)
                # wb transposes for group (i-11)//4
                if i % 4 == 3 and i >= 11 and (i - 11) // 4 < G:
                    gw = (i - 11) // 4
                    tensor.wait_ge(s_soft, gw + 1)
                    tensor.wait_ge(s_wtc, gw)  # pt WAR vs wt-copy(g-1)
                    for c, (s0, rows, _) in enumerate(S_CHUNKS):
                        ins = tensor.transpose(
                            pt[:rows, c * 97:c * 97 + 97],
                            wb[gw % 4][:, s0:s0 + rows],
                            id_sb[:97, :97],
                        )
                        if c == 2:
                            ins.then_inc(s_wtp, 1)
                # context matmuls for group (i-13)//4
                if i % 4 == 1 and i >= 13 and (i - 13) // 4 < G:
                    gc = (i - 13) // 4
                    tensor.wait_ge(s_wtc, gc + 1)
                    if gc >= 1:
                        tensor.wait_ge(s_csc, gc)  # pc WAR
                    for jj in range(4):
                        bg = 4 * gc + jj
                        for c, (s0, rows, _) in enumerate(S_CHUNKS):
                            ins = tensor.matmul(
                                pc[32 * jj:32 * jj + 1, :],
                                lhsT=wt[gc % 4][:rows, c * 97 + 32 * jj:c * 97 + 32 * jj + 1],
                                rhs=fb[bg % NB][:rows, c * EMB:(c + 1) * EMB],
                                start=(c == 0),
                                stop=(c == 2),
                                tile_position=(0, 32 * jj),
                                skip_group_check=True,
                            )
                            if jj == 3 and c == 2:
                                ins.then_inc(s_ctx, 1)

    return nc


_NC_CACHE = None


def _get_nc():
    global _NC_CACHE
    if _NC_CACHE is None:
        _NC_CACHE = build_nc()
    return _NC_CACHE


def _make_in_maps(inputs):
    hidden = np.asarray(inputs["hidden"], np.float32)
    features = np.asarray(inputs["features"], np.float32)
    w1w = np.asarray(inputs["W1_w"], np.float32)
    w1b = np.asarray(inputs["W1_b"], np.float32)
    w2w = np.asarray(inputs["W2_w"], np.float32)
    w2b = np.asarray(inputs["W2_b"], np.float32)
    vw = np.asarray(inputs["V_w"], np.float32)

    w1t = np.ascontiguousarray(w1w.T)
    w2t = np.ascontiguousarray(w2w.T)
    vby = np.zeros((128, 12), np.float32)
    vby[:, 0:4] = vw.reshape(4, 128).T
    vby[:, 4:8] = w1b.reshape(4, 128).T
    vby[:, 8:12] = w2b.reshape(4, 128).T
    ident = np.eye(128, dtype=np.float32)

    hid2 = hidden.reshape(BS, HID)
    in_maps = []
    for i in range(N_CORES):
        sl = slice(i * B, (i + 1) * B)
        in_maps.append(
            {
                "features": np.ascontiguousarray(features[sl]),
                "hidden": np.ascontiguousarray(hid2[sl]),
                "w1t": w1t,
                "w2t": w2t,
                "vby": vby,
                "ident": ident,
            }
        )
    return in_maps


def run(inputs, trace=False):
    nc = _get_nc()
    in_maps = _make_in_maps(inputs)
    res = run_bass_kernel_spmd(nc, in_maps, core_ids=list(range(N_CORES)), trace=trace)
    outs = [res.results[i]["out"] for i in range(N_CORES)]
    full = np.concatenate(outs, axis=0)
    ctx_v = np.ascontiguousarray(full[:, :EMB])
    attw = np.ascontiguousarray(full[:, EMB:]).reshape(BS, SEQ, 1)
    return (ctx_v, attw), res.exec_time_ns


def kernel(**inputs):
    (ctx_v, attw), _ = run(inputs, trace=False)
    return ctx_v, attw
